# revision 1
# baseline (speedup 1.0000x reference)
"""Per-pixel dynamic 5x5 conv (KernelConv2d) + leaky-relu, data-parallel on 8 TRN2 cores.

Sharding: core i <- (n = i//2, h-half = i%2); each core computes out[n, :, h0:h0+128, :].

v8 design (TensorE-accumulate, full-width products, HBM-roofline stream):
- DVE computes ONLY the 25 per-tap elementwise products (x-window * kernel
  plane) in x-row partition space; the dy partition-shift and the 25-tap sum
  run on the otherwise-idle TensorE as shifted-identity matmuls accumulating
  in fp32 PSUM (ldweights skipped when consecutive matmuls share weights).
- The input stream is at the device HBM roofline (8 cores x ~0.35 B/ns), so
  bytes are trimmed everywhere: x loaded once (even alignment; the odd
  1-col-shifted copy for 4B-aligned odd-dx DVE reads is built on-chip by
  ScalarE), the 5 shift matrices are built on-chip by DVE (memset +
  affine_select) during the DMA head, and the tail scatter matrix rides the
  tl DMA. ~13.5MB HBM per core vs baseline ~19MB.
- PE warm-up: dummy matmuls on garbage SBUF right after the preamble flip the
  HAM clock-gate to 2.4GHz before real work arrives.
- Tail: the last two kernel planes arrive as single-tap DMAs (tap 23's
  product overlaps tap 24's DMA); per 512-col PSUM chunk, stop-matmul ->
  ScalarE 0.2*x -> DVE max(0.2x, x) -> chunked output DMA on the ACT HWDGE
  ring (the sync ring would FIFO the output behind the whole input stream).
- Output rows p with p+dy > 127 (x rows 128..131) are covered by a
  50-partition host-gathered tail product + one scatter matmul, mid-queue.

Partition layout: partition q = padded x row h0+q (q=0..127). Product plane for
tap (dy,dx): P[q] = x[q, w+dx] * k[dy,dx][row h0+q-dy] (kernel plane pre-shifted
host-side, zero rows where q<dy). Shift_dy[q, p] = 1 iff p == q-dy routes row q
to output row p and matmul-accumulates over taps.
"""

import os
from contextlib import ExitStack

import numpy as np

import concourse.bass as bass
import concourse.mybir as mybir
from concourse.bass_utils import run_bass_kernel_spmd

N, C, H, W = 4, 8, 256, 256
K = 5
PAD = 2
NCORES = 8
HSH = H // 2            # 128 output rows per core
XW = 264                # stored x row width per alignment copy
CD = mybir.dt.float16
NEG = 0.2
NB = 8                  # product ring buffers
NWARM = 12              # PE warm-up dummy matmuls (HAM clock-gate)
NTAP = K * K            # 25
COMBOS = [(dy, p) for dy in (1, 2, 3, 4) for p in range(HSH - dy, HSH)]
NTAIL = len(COMBOS) * K  # 50
TAILPOS = 13            # queue position of the tail product (after product 12)
CW = C * W               # 2048
NKD = 14                 # kernel-plane DMAs: [0,1],(1,3),..,(21,23),[23],[24]

_NC_CACHE = {}


def _qpos(p):  # queue position of product p (tail occupies TAILPOS)
    return p if p < TAILPOS else p + 1


def _kdma(j):  # taps [t0, t1) carried by kernel DMA j
    if j == 0:
        return 0, 1
    if j <= 11:
        return 2 * j - 1, 2 * j + 1
    return j + 11, j + 12  # j=12 -> tap 23, j=13 -> tap 24


def _kdma_of(p):  # kernel DMA index carrying tap p
    if p == 0:
        return 0
    if p <= 22:
        return (p + 1) // 2
    return p - 11


def _build_nc():
    nc = bass.Bass("TRN2", target_bir_lowering=False, debug=False,
                   num_devices=NCORES)
    xe_d = nc.dram_tensor("xe", [HSH, C, XW], CD, kind="ExternalInput").ap()
    tl_d = nc.dram_tensor("tl", [NTAIL, 2 * CW + HSH], CD,
                          kind="ExternalInput").ap()
    kp_d = nc.dram_tensor("kp", [HSH, NTAP, C, W], CD, kind="ExternalInput").ap()
    out_d = nc.dram_tensor("out", [HSH, C, W], CD, kind="ExternalOutput").ap()

    with ExitStack() as ctx:
        xe = ctx.enter_context(nc.sbuf_tensor("xe_s", [HSH, C, XW], CD))
        xo = ctx.enter_context(nc.sbuf_tensor("xo_s", [HSH, C, XW], CD))
        wt = ctx.enter_context(nc.sbuf_tensor("wt_s", [HSH, K, HSH], CD))
        tl = ctx.enter_context(nc.sbuf_tensor("tl_s", [NTAIL, 2 * CW + HSH], CD))
        kt = ctx.enter_context(nc.sbuf_tensor("kt_s", [HSH, NTAP, C, W], CD))
        prod = [ctx.enter_context(nc.sbuf_tensor(f"pr{b}", [HSH, C, W], CD))
                for b in range(NB)]
        ptail = ctx.enter_context(nc.sbuf_tensor("ptail", [NTAIL, C, W], CD))
        tmp = ctx.enter_context(nc.sbuf_tensor("tmp", [HSH, C, W], CD))
        ot = ctx.enter_context(nc.sbuf_tensor("ot", [HSH, C, W], CD))
        al = ctx.enter_context(nc.sbuf_tensor("al", [HSH, 1], mybir.dt.float32))
        pt = ctx.enter_context(nc.psum_tensor("pt", [HSH, C, W], mybir.dt.float32))
        scr = ctx.enter_context(nc.psum_tensor("scr", [HSH, 512], mybir.dt.float32))

        xt = tl[:, 0:CW].rearrange("p (c w) -> p c w", c=C)
        ktl = tl[:, CW:2 * CW].rearrange("p (c w) -> p c w", c=C)
        wtl = tl[:, 2 * CW:]                      # [50, 128] scatter matrix

        s_xe = ctx.enter_context(nc.semaphore("s_xe"))
        s_tl = ctx.enter_context(nc.semaphore("s_tl"))
        s_k = [ctx.enter_context(nc.semaphore(f"s_k{j}")) for j in range(NKD)]
        s_w2 = ctx.enter_context(nc.semaphore("s_w2"))  # shift matrices built
        s_x2 = ctx.enter_context(nc.semaphore("s_x2"))  # odd copy done
        s_v = ctx.enter_context(nc.semaphore("s_v"))    # queue items produced
        s_mm = ctx.enter_context(nc.semaphore("s_mm"))  # queue items consumed
        s_c = ctx.enter_context(nc.semaphore("s_c"))    # per-chunk stop MMs
        s_t = ctx.enter_context(nc.semaphore("s_t"))    # 0.2*x chunks done
        s_e = ctx.enter_context(nc.semaphore("s_e"))    # lrelu chunks done
        s_o = ctx.enter_context(nc.semaphore("s_o"))
        block = ctx.enter_context(nc.Block())

        @block.sync
        def _(sync):
            sync.dma_start(xe[:], xe_d).then_inc(s_xe, 16)
            for j in range(NKD):
                t0, t1 = _kdma(j)
                sync.dma_start(kt[:, t0:t1], kp_d[:, t0:t1]).then_inc(s_k[j], 16)
                if j == 6:
                    sync.dma_start(tl[:], tl_d).then_inc(s_tl, 16)
            sync.wait_ge(s_o, 64)

        @block.gpsimd
        def _(gpsimd):
            # build the 5 shift matrices during the DMA head: wt[q, dy, p] =
            # 1 iff p == q - dy  (iota = dy + p - q, select where == 0)
            gpsimd.memset(al[:], NEG)  # leaky-relu slope for ActE alpha
            gpsimd.memset(wt[:], 1.0)
            for dy in range(K):
                sel = gpsimd.affine_select(wt[:, dy], wt[:, dy],
                                           pattern=[[1, HSH]], base=dy,
                                           channel_multiplier=-1,
                                           compare_op=mybir.AluOpType.is_equal,
                                           fill=0.0)
            sel.then_inc(s_w2, 1)

        @block.vector
        def _(vector):
            vector.wait_ge(s_xe, 16)
            for p in range(NTAP):
                dy, dx = divmod(p, K)
                a = dx & 1
                xsrc = xo if a else xe
                off = dx - a
                if p == 1:
                    vector.wait_ge(s_x2, 1)
                if p == 0 or (p % 2 == 1 and p <= 21) or p >= 23:
                    vector.wait_ge(s_k[_kdma_of(p)], 16)
                if p >= NB and p % 4 == 0:
                    # batched ring-reuse wait covering products p..p+3
                    vector.wait_ge(s_mm, _qpos(p - 5) + 1)
                vector.tensor_tensor(prod[p % NB][:],
                                     xsrc[:, :, off:off + W],
                                     kt[:, p],
                                     op=mybir.AluOpType.mult).then_inc(s_v, 1)
                if p == TAILPOS - 1:  # tail product right after product 12
                    vector.wait_ge(s_tl, 16)
                    vector.tensor_tensor(ptail[0:NTAIL], xt[0:NTAIL],
                                         ktl[0:NTAIL],
                                         op=mybir.AluOpType.mult).then_inc(s_v, 1)
            for q in range(4):
                # leaky-relu finish: max(0.2x from ScalarE, x from PSUM)
                vector.wait_ge(s_t, q + 1)
                vector.tensor_tensor(ot[:, 2 * q:2 * q + 2],
                                     tmp[:, 2 * q:2 * q + 2],
                                     pt[:, 2 * q:2 * q + 2],
                                     op=mybir.AluOpType.max).then_inc(s_e, 1)

        @block.tensor
        def _(tensor):
            # warm-up on garbage SBUF (ot is only written much later, and that
            # write is sem-ordered after these reads) -- no DMA dependency, so
            # the PE is busy right out of the preamble and HAM unthrottles
            for r in range(NWARM):
                mm = tensor.matmul(scr[:], lhsT=ot[:, 0, 0:HSH],
                                   rhs=ot[:, 0:2, :], start=True, stop=True)
                if r > 0:
                    mm.ins.ldweights = False
            tensor.wait_ge(s_w2, 1)
            prev_w = [-1]

            def mmul(rhs_ap, q, wid, start, stop):
                mm = tensor.matmul(pt[:, 2 * q:2 * q + 2],
                                   lhsT=(wtl[0:NTAIL] if wid == 5
                                         else wt[:, wid]),
                                   rhs=rhs_ap,
                                   start=start, stop=stop)
                if wid == prev_w[0]:
                    mm.ins.ldweights = False
                prev_w[0] = wid
                return mm

            for n in range(NTAP + 1):  # queue: products + tail at TAILPOS
                if n == TAILPOS:
                    tensor.wait_ge(s_v, n + 1)
                    for q in range(4):
                        mm = mmul(ptail[0:NTAIL, 2 * q:2 * q + 2], q, 5,
                                  False, False)
                    mm.then_inc(s_mm, 1)
                    continue
                p = n if n < TAILPOS else n - 1
                dy = p // K
                first, last = p == 0, p == NTAP - 1
                tensor.wait_ge(s_v, n + 1)
                for q in range(4):
                    mm = mmul(prod[p % NB][:, 2 * q:2 * q + 2], q, dy,
                              first, last)
                    if last:
                        mm.then_inc(s_c, 1)
                if not last:
                    mm.then_inc(s_mm, 1)

        @block.scalar
        def _(scalar):
            # odd-alignment x copy: xo[i] = xe[i+1] (cuts 0.54MB off the
            # DMA stream; also preloads the ACT table)
            scalar.wait_ge(s_xe, 16)
            scalar.activation(xo[:, :, 0:XW - 4], xe[:, :, 1:XW - 3],
                              mybir.ActivationFunctionType.Copy,
                              bias=0.0, scale=1.0).then_inc(s_x2, 1)
            # evict: 0.2*x prep per chunk (Lrelu's alpha is dropped by the
            # lowering -- as immediate AND as AP -- so lrelu is act+max), then
            # output DMAs on the ACT HWDGE ring (sync's would FIFO behind the
            # whole input stream)
            for q in range(4):
                scalar.wait_ge(s_c, q + 1)
                scalar.activation(tmp[:, 2 * q:2 * q + 2],
                                  pt[:, 2 * q:2 * q + 2],
                                  mybir.ActivationFunctionType.Copy,
                                  bias=0.0, scale=NEG).then_inc(s_t, 1)
            for q in range(4):
                scalar.wait_ge(s_e, q + 1)
                scalar.dma_start(out_d[:, 2 * q:2 * q + 2],
                                 ot[:, 2 * q:2 * q + 2]).then_inc(s_o, 16)
    return nc


def get_nc():
    if "nc" not in _NC_CACHE:
        _NC_CACHE["nc"] = _build_nc()
    return _NC_CACHE["nc"]


def _prep_shards(x: np.ndarray, kernel: np.ndarray):
    """Host-side: pad, cast to fp16, build per-core DMA layouts."""
    f16 = np.float16
    xp = np.pad(x, ((0, 0), (0, 0), (PAD, PAD), (PAD, XW + 1 - W - PAD)),
                mode='edge').astype(f16)  # (N, C, 260, 265)
    kr = kernel.reshape(N, C, NTAP, H, W)

    in_maps = []
    for core in range(NCORES):
        n, hb = divmod(core, 2)
        h0 = hb * HSH
        blk = xp[n, :, h0:h0 + HSH + 4, :]          # (C, 132, 265)
        xeb = np.ascontiguousarray(blk[:, :HSH, 0:XW].transpose(1, 0, 2))

        kb = kr[n, :, :, h0:h0 + HSH, :].astype(f16)  # (C, 25, 128, W)
        kp = np.zeros((HSH, NTAP, C, W), f16)
        for t in range(NTAP):
            dy = t // K
            kp[dy:, t] = kb[:, t, :HSH - dy].transpose(1, 0, 2)

        tlb = np.zeros((NTAIL, 2 * CW + HSH), f16)
        xtv = tlb[:, 0:CW].reshape(NTAIL, C, W)
        ktv = tlb[:, CW:2 * CW].reshape(NTAIL, C, W)
        wtv = tlb[:, 2 * CW:]
        for j, (dy, p) in enumerate(COMBOS):
            for dx in range(K):
                xtv[j * K + dx] = blk[:, p + dy, dx:dx + W]
                ktv[j * K + dx] = kb[:, dy * K + dx, p]
                wtv[j * K + dx, p] = 1.0

        in_maps.append({"xe": xeb, "tl": tlb, "kp": kp})
    return in_maps


def kernel(x: np.ndarray, kernel: np.ndarray) -> np.ndarray:
    nc = get_nc()
    in_maps = _prep_shards(np.asarray(x), np.asarray(kernel))
    trace = bool(int(os.environ.get("KC_TRACE", "0")))
    res = run_bass_kernel_spmd(nc, in_maps, core_ids=list(range(NCORES)),
                               trace=trace)
    _NC_CACHE["last_results"] = res
    out = np.empty((N, C, H, W), np.float32)
    for core in range(NCORES):
        n, hb = divmod(core, 2)
        h0 = hb * HSH
        o = res.results[core]["out"]  # (128, C, W) fp16
        out[n, :, h0:h0 + HSH, :] = o.transpose(1, 0, 2).astype(np.float32)
    return out

